# revision 12
# baseline (speedup 1.0000x reference)
"""HSMM forward-pass kernel for Trainium2 (8 NeuronCores, data-parallel over batch).

Algorithm: the explicit-duration HSMM forward recurrence is rewritten in
exp-space, where it becomes a *linear* recurrence with time-varying diagonal
scalings:

    w_t = EC2_t * ( expA^T @ (EC1_t * sum_{d=1..64} expD_d * w_{t-d}) )

with all EC* factors precomputable on the host from cumsum(log_B).  Numerical
range is controlled by per-j anchors Phi_n(j) = cumb at 64-step block
boundaries (host precomputed) plus one runtime scalar per block per sequence
(mu_n), estimated on-device with a partition-sum probe and applied as a ring
rescale at each block boundary.  alpha_t = log(G_t) + cumb_t - Phi_n + mu_n
is reconstructed afterwards (log on device, the rest on host).

Each core handles 2 sequences; K=256 states are split into 2 partition-halves
(g = 2*h + b indexes the 4 (half, seq) groups laid out along the free dim).
The 64-tap duration filter runs on the Vector engine against a linear (non
circular) ring buffer that is repacked/rescaled once per 64-step block; the
transition matvec runs on the Tensor engine with bf16 weights.
"""

import os
import sys

import numpy as np

sys.path.insert(0, "/opt/trn_rl_repo")

import ml_dtypes

BF = ml_dtypes.bfloat16

B, T, K, DMAX = 16, 2048, 256, 64
NCORES = 8
SPC = B // NCORES  # sequences per core = 2
BLK = 64  # block length (= DMAX)
PROBE_K = 32  # probe offset within a block
HCB = 133  # ring slot base of the persistent hc region


# ---------------------------------------------------------------- host precompute
def _precompute(log_B, pi_logits, A_logits, D_logits, t_steps=T):
    """Build per-core device input arrays + host-side reconstruction arrays."""
    nblk = t_steps // BLK
    logb = np.asarray(log_B, dtype=np.float64)[:, :t_steps, :]
    cumb = np.cumsum(logb, axis=1)  # [B, t, K] float64

    # anchors Phi_n = cumb at block starts (cumb_{64n-1}; 0 for n=0)
    phi = np.zeros((B, nblk, K), dtype=np.float64)
    for n in range(1, nblk):
        phi[:, n, :] = cumb[:, BLK * n - 1, :]

    nidx = np.arange(t_steps) // BLK
    phit = phi[:, nidx, :]  # [B, t, K]
    ec1 = np.exp(cumb - phit)  # [B, t, K]
    ec2 = np.exp(phit - cumb)
    fr = np.ones((B, nblk, K), dtype=np.float64)
    if nblk > 1:
        fr[:, : nblk - 1, :] = np.exp(phi[:, 1:, :] - phi[:, : nblk - 1, :])
    expD1 = np.exp(np.asarray(D_logits, dtype=np.float64))[:, 0]  # [K]
    # sc[b, t, j, 0] = EC2_t ; sc[b, t, j, 1] = hc scale for step t+1
    sc = np.zeros((B, t_steps, K, 2), dtype=np.float64)
    sc[..., 0] = ec2
    # hc scale for t+1 is anchor-free: exp(logb_{t+1}) * expD(d=1)
    sc[:, : t_steps - 1, :, 1] = np.exp(logb[:, 1:, :]) * expD1[None, None, :]

    exppi = np.exp(np.asarray(pi_logits, dtype=np.float64))  # [K]
    expA = np.exp(np.asarray(A_logits, dtype=np.float64))  # [i, j]
    expD = np.exp(np.asarray(D_logits, dtype=np.float64))  # [j, d-1]

    # device layouts ------------------------------------------------------
    def to_dev(x_btj, dtype):
        # [b, t, K] -> [128p, g=(h,b), t]
        b_, t_, _ = x_btj.shape
        v = x_btj.reshape(b_, t_, 2, 128).transpose(3, 2, 0, 1)  # [p, h, b, t]
        return np.ascontiguousarray(v.reshape(128, 2 * b_, t_)).astype(dtype)

    # edrevF: u=0 -> 0 (d=65 pad); u>=1 -> expD col 64-u (d = 65-u in 2..64)
    edrev = np.zeros((128, 4, DMAX), dtype=np.float64)
    for h in range(2):
        for b in range(2):
            edrev[:, 2 * h + b, 1:] = expD[h * 128 : (h + 1) * 128, ::-1][:, 0 : DMAX - 1]
    wexp = np.empty((128, 2, 256), dtype=np.float64)
    for hi in range(2):
        wexp[:, hi, :] = expA[hi * 128 : (hi + 1) * 128, :]

    # ring layout: slots 0..128 history (slot u <-> tau = 64n-65+u),
    # 131/132 hfar double-buffer, HCB+t = hc_t (persistent, output-needed)
    rs = HCB + t_steps + 1
    ring0 = np.zeros((NCORES, 128, 4, rs), dtype=np.float64)
    cumb0 = cumb[:, 0, :]  # [B, K]
    for c in range(NCORES):
        for h in range(2):
            for b in range(2):
                gb = SPC * c + b
                ring0[c, :, 2 * h + b, BLK] = exppi[h * 128 : (h + 1) * 128]
                ring0[c, :, 2 * h + b, HCB] = (
                    exppi[h * 128 : (h + 1) * 128]
                    * expD1[h * 128 : (h + 1) * 128]
                    * np.exp(cumb0[gb, h * 128 : (h + 1) * 128])
                )

    per_core = []
    for c in range(NCORES):
        bs = slice(SPC * c, SPC * (c + 1))
        scc = sc[bs].reshape(SPC, t_steps, 2, 128, 2).transpose(3, 2, 0, 1, 4)
        scc = np.ascontiguousarray(scc.reshape(128, 2 * SPC, t_steps, 2))
        per_core.append(
            dict(
                ec1=to_dev(ec1[bs], BF),
                ec1x=to_dev(ec1[bs] * (2.0**-40), np.float32),
                sc=scc.astype(BF),
                fr=to_dev(fr[bs], BF),
                ring0=ring0[c].astype(BF),
                edrev=edrev.astype(BF),
                wexp=wexp.astype(BF),
                ones_k=np.ones((128, 1), dtype=BF),
                ones_m=np.ones((1, 128), dtype=np.float32),
            )
        )
    host = dict(nblk=nblk)
    return per_core, host


# ---------------------------------------------------------------- device kernel
def _build(t_steps=T, debug=False):
    import concourse.bass as bass
    import concourse.mybir as mybir
    from concourse import bacc, tile

    fp32 = mybir.dt.float32
    bf16 = mybir.dt.bfloat16
    MUL = mybir.AluOpType.mult
    nblk = t_steps // BLK

    nc = bacc.Bacc("TRN2", target_bir_lowering=False, debug=debug)

    ec1_d = nc.dram_tensor("ec1", [128, 4, t_steps], bf16, kind="ExternalInput")
    ec1x_d = nc.dram_tensor("ec1x", [128, 4, t_steps], fp32, kind="ExternalInput")
    sc_d = nc.dram_tensor("sc", [128, 4, t_steps, 2], bf16, kind="ExternalInput")
    fr_d = nc.dram_tensor("fr", [128, 4, nblk], bf16, kind="ExternalInput")
    rs = HCB + t_steps + 1
    ring0_d = nc.dram_tensor("ring0", [128, 4, rs], bf16, kind="ExternalInput")
    edrev_d = nc.dram_tensor("edrev", [128, 4, DMAX], bf16, kind="ExternalInput")
    wexp_d = nc.dram_tensor("wexp", [128, 2, 256], bf16, kind="ExternalInput")
    ones_k_d = nc.dram_tensor("ones_k", [128, 1], bf16, kind="ExternalInput")
    ones_m_d = nc.dram_tensor("ones_m", [1, 128], fp32, kind="ExternalInput")

    araw_d = nc.dram_tensor("alpha_raw", [128, 4, t_steps], fp32, kind="ExternalOutput")
    sprb_d = nc.dram_tensor("sprobe", [1, 2, nblk], fp32, kind="ExternalOutput")

    with tile.TileContext(nc) as tc:
        with (
            tc.tile_pool(name="persist", bufs=1) as pp,
            tc.tile_pool(name="work", bufs=3) as wp,
            tc.tile_pool(name="psum", bufs=2, space="PSUM") as pfp,
            tc.tile_pool(name="psum_s", bufs=2, space="PSUM") as psp,
        ):
            ec1 = pp.tile([128, 4, t_steps], bf16, tag="ec1")
            ec1x = pp.tile([128, 4, t_steps], fp32, tag="ec1x")
            sc = pp.tile([128, 4, t_steps, 2], bf16, tag="sc")
            fr = pp.tile([128, 4, nblk], bf16, tag="fr")
            ring = pp.tile([128, 4, rs], bf16, tag="ring")
            edrev = pp.tile([128, 4, DMAX], bf16, tag="edrev")
            wexp = pp.tile([128, 2, 256], bf16, tag="wexp")
            ones_k = pp.tile([128, 1], bf16, tag="ones_k")
            ones_m = pp.tile([1, 128], fp32, tag="ones_m")
            gbuf = pp.tile([128, 4, t_steps], fp32, tag="gbuf")
            sout = pp.tile([1, 2, nblk], fp32, tag="sout")
            inv_s = pp.tile([1, 2], fp32, tag="inv_s")

            nc.sync.dma_start(ec1[:], ec1_d[:])
            nc.sync.dma_start(ec1x[:], ec1x_d[:])
            nc.sync.dma_start(sc[:], sc_d[:])
            nc.sync.dma_start(fr[:], fr_d[:])
            nc.sync.dma_start(ring[:], ring0_d[:])
            nc.sync.dma_start(edrev[:], edrev_d[:])
            nc.sync.dma_start(wexp[:], wexp_d[:])
            nc.sync.dma_start(ones_k[:], ones_k_d[:])
            nc.sync.dma_start(ones_m[:], ones_m_d[:])
            nc.vector.memset(sout[:], 0.0)

            for t in range(t_steps):
                n, k = divmod(t, BLK)
                hslot = 131 + (t % 2)
                hcslot = HCB + t
                # far duration filter (d=2..64) on GpSimd; window slots k..k+63
                prod = wp.tile([128, 4, DMAX], bf16, tag="prod")
                nc.gpsimd.tensor_mul(prod[:], ring[:, :, k : k + DMAX], edrev[:])
                # reduce: groups 0-1 on Vector, 2-3 on Scalar (activation accum)
                nc.vector.tensor_reduce(
                    gbuf[:, 0:2, t],
                    prod[:, 0:2, :],
                    axis=mybir.AxisListType.X,
                    op=mybir.AluOpType.add,
                )
                scr = wp.tile([128, 2, DMAX], bf16, tag="scr")
                nc.scalar.activation(
                    scr[:, 0, :],
                    prod[:, 2, :],
                    mybir.ActivationFunctionType.Copy,
                    accum_out=gbuf[:, 2:3, t],
                )
                nc.scalar.activation(
                    scr[:, 1, :],
                    prod[:, 3, :],
                    mybir.ActivationFunctionType.Copy,
                    accum_out=gbuf[:, 3:4, t],
                )
                # hfar column
                nc.vector.tensor_mul(ring[:, :, hslot], gbuf[:, :, t], ec1[:, :, t])
                # transition matvec: f = expA^T @ (hfar + hc), 8 matmuls
                pf = pfp.tile([128, 4], fp32, tag="pf")
                for hj in range(2):
                    for hi in range(2):
                        w_ap = wexp[:, hi, hj * 128 : (hj + 1) * 128]
                        nc.tensor.matmul(
                            pf[:, 2 * hj : 2 * hj + 2],
                            w_ap,
                            ring[:, 2 * hi : 2 * hi + 2, hslot],
                            start=(hi == 0),
                            stop=False,
                        )
                        nc.tensor.matmul(
                            pf[:, 2 * hj : 2 * hj + 2],
                            w_ap,
                            ring[:, 2 * hi : 2 * hi + 2, hcslot],
                            start=False,
                            stop=(hi == 1),
                        )
                # opA: w~ -> ring slot 65+k, hc for t+1 -> slot HCB+t+1
                base = 65 + k
                second = HCB + t + 1
                out_ap = ring[:, :, base : second + 1 : second - base]
                nc.vector.tensor_mul(
                    out_ap,
                    pf[:].unsqueeze(2).broadcast_to([128, 4, 2]),
                    sc[:, :, t, :],
                )

                if k == PROBE_K and n < nblk - 1:
                    # probe: S[b] ~= sum_j hfar (both halves accumulate in psum)
                    ps = psp.tile([1, 2], fp32, tag="ps")
                    nc.tensor.matmul(
                        ps[:], ones_k[:], ring[:, 0:2, hslot], start=True, stop=False
                    )
                    nc.tensor.matmul(
                        ps[:], ones_k[:], ring[:, 2:4, hslot], start=False, stop=True
                    )
                    nc.vector.tensor_copy(sout[:, :, n], ps[0:1, :])
                    nc.vector.reciprocal(inv_s[:], ps[0:1, :])

                if k == BLK - 1 and n < nblk - 1:
                    # block boundary: rescale+repack ring slots 64..128 -> 0..64
                    pb = psp.tile([128, 4], fp32, tag="pb")
                    rhs_b = inv_s[0:1, :].unsqueeze(1).broadcast_to([1, 2, 2])
                    nc.tensor.matmul(pb[:], ones_m[:], rhs_b, start=True, stop=True)
                    frmu = wp.tile([128, 4], bf16, tag="frmu")
                    nc.vector.tensor_mul(frmu[:], fr[:, :, n], pb[:])
                    nc.vector.tensor_mul(
                        ring[:, :, 0:65],
                        ring[:, :, 64:129],
                        frmu[:].unsqueeze(2).broadcast_to([128, 4, 65]),
                    )
                    nc.gpsimd.memset(ring[:, :, 65:129], 0.0)
                    # pending hc (slot HCB+t+1) needs the mu rescale too
                    nc.vector.tensor_mul(
                        ring[:, :, HCB + t + 1], ring[:, :, HCB + t + 1], pb[:]
                    )

            # alpha-centered output: G_far*EC1*2^-40 + hc*2^-40, then log
            nc.vector.tensor_mul(gbuf[:], gbuf[:], ec1x[:])
            nc.vector.scalar_tensor_tensor(
                gbuf[:],
                ring[:, :, HCB : HCB + t_steps],
                2.0**-40,
                gbuf[:],
                op0=mybir.AluOpType.mult,
                op1=mybir.AluOpType.add,
            )
            nc.scalar.activation(
                gbuf[:], gbuf[:], mybir.ActivationFunctionType.Ln
            )
            nc.sync.dma_start(araw_d[:], gbuf[:])
            nc.sync.dma_start(sprb_d[:], sout[:])

    nc.compile()
    return nc


# ---------------------------------------------------------------- golden numpy sim
def _golden(core_in, t_steps=T):
    """Replicates device ops (incl. bf16 rounding) for one core in numpy."""
    nblk = t_steps // BLK
    f32 = np.float32

    def bf(x):
        return x.astype(BF).astype(f32)

    ring = core_in["ring0"].astype(f32)  # [128, 4, 133]
    ec1x = core_in["ec1x"].astype(f32)
    edrev = core_in["edrev"].astype(f32)
    wexp = core_in["wexp"].astype(f32)
    ec1 = core_in["ec1"].astype(f32)
    sc = core_in["sc"].astype(f32)
    fr = core_in["fr"].astype(f32)
    gbuf = np.zeros((128, 4, t_steps), f32)
    sout = np.zeros((1, 2, nblk), f32)

    for t in range(t_steps):
        n, k = divmod(t, BLK)
        hslot = 131 + (t % 2)
        hcslot = HCB + t
        prod = bf(ring[:, :, k : k + DMAX] * edrev)
        g = prod.sum(axis=2, dtype=f32)
        gbuf[:, :, t] = g
        hfar = bf(g * ec1[:, :, t])
        ring[:, :, hslot] = hfar
        hc = ring[:, :, hcslot]
        pf = np.zeros((128, 4), f32)
        for hj in range(2):
            for b in range(2):
                acc = np.zeros(128, f32)
                for hi in range(2):
                    w = wexp[:, hi, hj * 128 : (hj + 1) * 128]
                    acc += w.T @ hfar[:, 2 * hi + b] + w.T @ hc[:, 2 * hi + b]
                pf[:, 2 * hj + b] = acc
        ring[:, :, 65 + k] = bf(pf * sc[:, :, t, 0])
        ring[:, :, HCB + t + 1] = bf(pf * sc[:, :, t, 1])
        if k == PROBE_K and n < nblk - 1:
            s4 = hfar.sum(axis=0, dtype=f32)
            sout[0, :, n] = s4[0:2] + s4[2:4]
        if k == BLK - 1 and n < nblk - 1:
            inv = (1.0 / sout[0, :, n]).astype(f32)
            frmu = bf(fr[:, :, n] * inv[None, [0, 1, 0, 1]])
            newhist = bf(ring[:, :, 64:129] * frmu[:, :, None])
            ring[:, :, 0:65] = newhist
            ring[:, :, 65:129] = 0.0
            ring[:, :, HCB + t + 1] = bf(
                ring[:, :, HCB + t + 1] * inv[None, [0, 1, 0, 1]]
            )
    gtot = gbuf * ec1x + ring[:, :, HCB : HCB + t_steps] * np.float32(2.0**-40)
    return np.log(gtot), sout


# ---------------------------------------------------------------- host postprocess
def _postprocess(raw_list, sp_list, t_steps=T):
    """raw: [128, 4, t] log(G*EC1*2^-40) per core; sp: [1, 2, nblk] probe sums."""
    nblk = t_steps // BLK
    shift = np.float32(40.0 * np.log(2.0))
    alphas = np.empty((B, t_steps, K), dtype=np.float32)
    for c in range(NCORES):
        raw = np.asarray(raw_list[c])  # [128, 4, t]
        sp = np.asarray(sp_list[c])  # [1, 2, nblk]
        logs = np.zeros((2, nblk), dtype=np.float64)
        logs[:, 1:] = np.log(np.maximum(sp[0, :, : nblk - 1], 1e-300))
        mu = (np.cumsum(logs, axis=1) + 40.0 * np.log(2.0)).astype(np.float32)
        nidx = np.arange(t_steps) // BLK
        for b in range(2):
            gb = SPC * c + b
            # [p, h, t] -> [t, h*128+p]
            lg = raw[:, [2 * 0 + b, 2 * 1 + b], :].transpose(2, 1, 0).reshape(t_steps, K)
            alphas[gb] = lg + mu[b, nidx][:, None]
    last = alphas[:, -1, :].astype(np.float64)
    m = last.max(axis=1)
    loglik = (np.log(np.exp(last - m[:, None]).sum(axis=1)) + m).astype(np.float32)
    return loglik, alphas


# ---------------------------------------------------------------- entry point
_CACHE = {}


def _run(log_B, pi_logits, A_logits, D_logits, trace=False, trace_kwargs=None):
    from concourse.bass_utils import run_bass_kernel_spmd

    t_steps = T
    per_core, host = _precompute(log_B, pi_logits, A_logits, D_logits, t_steps)
    if "nc" not in _CACHE:
        _CACHE["nc"] = _build(t_steps, debug=False)
    nc = _CACHE["nc"]
    in_maps = [per_core[c] for c in range(NCORES)]
    out = run_bass_kernel_spmd(
        nc, in_maps, list(range(NCORES)), trace=trace, **(trace_kwargs or {})
    )
    res = out.results
    raw_list = [res[c]["alpha_raw"] for c in range(NCORES)]
    sp_list = [res[c]["sprobe"] for c in range(NCORES)]
    loglik, alphas = _postprocess(raw_list, sp_list, t_steps)
    return loglik, alphas, out


def kernel(log_B, pi_logits, A_logits, D_logits):
    loglik, alphas, _ = _run(log_B, pi_logits, A_logits, D_logits)
    return loglik, alphas


def _ensure_ntff_hook():
    """Register the axon NTFF profile hook if the image lacks antenv.axon_hooks."""
    import types

    try:
        from antenv.axon_hooks import get_axon_ntff_profile_hook  # noqa: F401

        return
    except ImportError:
        pass
    if "/root/.axon_site" not in sys.path:
        sys.path.insert(0, "/root/.axon_site")
    from trn_agent_boot.trn_boot import _ntff_profile_via_ctypes

    import antenv

    hook = _ntff_profile_via_ctypes("/opt/axon/libaxon_pjrt.so")
    mod = types.ModuleType("antenv.axon_hooks")
    holder = {"h": hook}
    mod.get_axon_ntff_profile_hook = lambda: holder["h"]
    mod.set_axon_ntff_profile_hook = lambda h: holder.__setitem__("h", h)
    sys.modules["antenv.axon_hooks"] = mod
    antenv.axon_hooks = mod


def profile_exec_ns(log_B, pi_logits, A_logits, D_logits, tmpdir=None):
    """Run with NTFF tracing; returns HW exec time in ns (or None)."""
    _ensure_ntff_hook()
    from concourse import bass_utils as _bu

    if not getattr(_bu.upload_artifacts, "_patched", False):
        def _no_upload(tmpdir_):
            return "local://" + str(tmpdir_)

        _no_upload._patched = True
        _bu.upload_artifacts = _no_upload
    kw = {"tmpdir": tmpdir} if tmpdir else {}
    _, _, out = _run(log_B, pi_logits, A_logits, D_logits, trace=True, trace_kwargs=kw)
    return out.exec_time_ns


# revision 14
# speedup vs baseline: 1.2477x; 1.2477x over previous
"""HSMM forward-pass kernel for Trainium2 (8 NeuronCores, data-parallel over batch).

Algorithm: the explicit-duration HSMM forward recurrence is rewritten in
exp-space, where it becomes a *linear* recurrence with time-varying diagonal
scalings:

    w_t = EC2_t * ( expA^T @ (EC1_t * sum_{d=1..64} expD_d * w_{t-d}) )

with all EC* factors precomputable on the host from cumsum(log_B).  Numerical
range is controlled by per-j anchors Phi_n(j) = cumb at 64-step block
boundaries (host precomputed) plus one runtime scalar per block per sequence
(mu_n), estimated on-device with a partition-sum probe and applied as a ring
rescale at each block boundary.  alpha_t = log(G_t) + cumb_t - Phi_n + mu_n
is reconstructed afterwards (log on device, the rest on host).

Each core handles 2 sequences; K=256 states are split into 2 partition-halves
(g = 2*h + b indexes the 4 (half, seq) groups laid out along the free dim).
The 64-tap duration filter runs on the Vector engine against a linear (non
circular) ring buffer that is repacked/rescaled once per 64-step block; the
transition matvec runs on the Tensor engine with bf16 weights.
"""

import os
import sys

import numpy as np

sys.path.insert(0, "/opt/trn_rl_repo")

import ml_dtypes

BF = ml_dtypes.bfloat16

B, T, K, DMAX = 16, 2048, 256, 64
NCORES = 8
SPC = B // NCORES  # sequences per core = 2
BLK = 64  # block length (= DMAX)
PROBE_K = 32  # probe offset within a block
HCB = 133  # ring slot base of the persistent hc region


# ---------------------------------------------------------------- host precompute
def _precompute(log_B, pi_logits, A_logits, D_logits, t_steps=T):
    """Build per-core device input arrays + host-side reconstruction arrays."""
    nblk = t_steps // BLK
    logb = np.asarray(log_B, dtype=np.float64)[:, :t_steps, :]
    cumb = np.cumsum(logb, axis=1)  # [B, t, K] float64

    # anchors Phi_n = cumb at block starts (cumb_{64n-1}; 0 for n=0)
    phi = np.zeros((B, nblk, K), dtype=np.float64)
    for n in range(1, nblk):
        phi[:, n, :] = cumb[:, BLK * n - 1, :]

    nidx = np.arange(t_steps) // BLK
    phit = phi[:, nidx, :]  # [B, t, K]
    ec1 = np.exp(cumb - phit)  # [B, t, K]
    ec2 = np.exp(phit - cumb)
    fr = np.ones((B, nblk, K), dtype=np.float64)
    if nblk > 1:
        fr[:, : nblk - 1, :] = np.exp(phi[:, 1:, :] - phi[:, : nblk - 1, :])
    expD1 = np.exp(np.asarray(D_logits, dtype=np.float64))[:, 0]  # [K]
    # sc[b, t, j, 0] = EC2_t ; sc[b, t, j, 1] = hc scale for step t+1
    sc = np.zeros((B, t_steps, K, 2), dtype=np.float64)
    sc[..., 0] = ec2
    # hc scale for t+1 is anchor-free: exp(logb_{t+1}) * expD(d=1)
    sc[:, : t_steps - 1, :, 1] = np.exp(logb[:, 1:, :]) * expD1[None, None, :]

    exppi = np.exp(np.asarray(pi_logits, dtype=np.float64))  # [K]
    expA = np.exp(np.asarray(A_logits, dtype=np.float64))  # [i, j]
    expD = np.exp(np.asarray(D_logits, dtype=np.float64))  # [j, d-1]

    # device layouts ------------------------------------------------------
    def to_dev(x_btj, dtype):
        # [b, t, K] -> [128p, g=(h,b), t]
        b_, t_, _ = x_btj.shape
        v = x_btj.reshape(b_, t_, 2, 128).transpose(3, 2, 0, 1)  # [p, h, b, t]
        return np.ascontiguousarray(v.reshape(128, 2 * b_, t_)).astype(dtype)

    # edrevF: u=0 -> 0 (d=65 pad); u>=1 -> expD col 64-u (d = 65-u in 2..64)
    edrev = np.zeros((128, 4, DMAX), dtype=np.float64)
    for h in range(2):
        for b in range(2):
            edrev[:, 2 * h + b, 1:] = expD[h * 128 : (h + 1) * 128, ::-1][:, 0 : DMAX - 1]
    wexp = np.empty((128, 2, 256), dtype=np.float64)
    for hi in range(2):
        wexp[:, hi, :] = expA[hi * 128 : (hi + 1) * 128, :]

    # ring layout: slots 0..128 history (slot u <-> tau = 64n-65+u),
    # 129/130 hc double-buffer, 131/132 hfar double-buffer
    ring0 = np.zeros((NCORES, 128, 4, 133), dtype=np.float64)
    cumb0 = cumb[:, 0, :]  # [B, K]
    for c in range(NCORES):
        for h in range(2):
            for b in range(2):
                gb = SPC * c + b
                ring0[c, :, 2 * h + b, BLK] = exppi[h * 128 : (h + 1) * 128]
                ring0[c, :, 2 * h + b, 129] = (
                    exppi[h * 128 : (h + 1) * 128]
                    * expD1[h * 128 : (h + 1) * 128]
                    * np.exp(cumb0[gb, h * 128 : (h + 1) * 128])
                )
    # ratio[tau] = EC1_tau * exp(logb_{tau+1}) * expD1 : recovers hc_{tau+1}
    # from the stored w~_tau once per block (batched output persistence)
    ratio = np.zeros((B, t_steps, K), dtype=np.float64)
    ratio[:, : t_steps - 1, :] = (
        ec1[:, : t_steps - 1, :] * np.exp(logb[:, 1:, :]) * expD1[None, None, :]
    )

    per_core = []
    for c in range(NCORES):
        bs = slice(SPC * c, SPC * (c + 1))
        scc = sc[bs].reshape(SPC, t_steps, 2, 128, 2).transpose(3, 2, 0, 1, 4)
        scc = np.ascontiguousarray(scc.reshape(128, 2 * SPC, t_steps, 2))
        per_core.append(
            dict(
                ec1=to_dev(ec1[bs], BF),
                ec1x=to_dev(ec1[bs] * (2.0**-40), np.float32),
                sc=scc.astype(BF),
                ratio=to_dev(ratio[bs], BF),
                fr=to_dev(fr[bs], BF),
                ring0=ring0[c].astype(BF),
                edrev=edrev.astype(BF),
                wexp=wexp.astype(BF),
                ones_k=np.ones((128, 1), dtype=BF),
                ones_m=np.ones((1, 128), dtype=np.float32),
            )
        )
    host = dict(nblk=nblk)
    return per_core, host


# ---------------------------------------------------------------- device kernel
def _build(t_steps=T, debug=False):
    import concourse.bass as bass
    import concourse.mybir as mybir
    from concourse import bacc, tile

    fp32 = mybir.dt.float32
    bf16 = mybir.dt.bfloat16
    MUL = mybir.AluOpType.mult
    nblk = t_steps // BLK

    nc = bacc.Bacc("TRN2", target_bir_lowering=False, debug=debug)

    ec1_d = nc.dram_tensor("ec1", [128, 4, t_steps], bf16, kind="ExternalInput")
    ec1x_d = nc.dram_tensor("ec1x", [128, 4, t_steps], fp32, kind="ExternalInput")
    sc_d = nc.dram_tensor("sc", [128, 4, t_steps, 2], bf16, kind="ExternalInput")
    fr_d = nc.dram_tensor("fr", [128, 4, nblk], bf16, kind="ExternalInput")
    ring0_d = nc.dram_tensor("ring0", [128, 4, 133], bf16, kind="ExternalInput")
    ratio_d = nc.dram_tensor("ratio", [128, 4, t_steps], bf16, kind="ExternalInput")
    edrev_d = nc.dram_tensor("edrev", [128, 4, DMAX], bf16, kind="ExternalInput")
    wexp_d = nc.dram_tensor("wexp", [128, 2, 256], bf16, kind="ExternalInput")
    ones_k_d = nc.dram_tensor("ones_k", [128, 1], bf16, kind="ExternalInput")
    ones_m_d = nc.dram_tensor("ones_m", [1, 128], fp32, kind="ExternalInput")

    araw_d = nc.dram_tensor("alpha_raw", [128, 4, t_steps], fp32, kind="ExternalOutput")
    sprb_d = nc.dram_tensor("sprobe", [1, 2, nblk], fp32, kind="ExternalOutput")

    with tile.TileContext(nc) as tc:
        with (
            tc.tile_pool(name="persist", bufs=1) as pp,
            tc.tile_pool(name="work", bufs=3) as wp,
            tc.tile_pool(name="psum", bufs=2, space="PSUM") as pfp,
            tc.tile_pool(name="psum_s", bufs=2, space="PSUM") as psp,
        ):
            ec1 = pp.tile([128, 4, t_steps], bf16, tag="ec1")
            ec1x = pp.tile([128, 4, t_steps], fp32, tag="ec1x")
            sc = pp.tile([128, 4, t_steps, 2], bf16, tag="sc")
            fr = pp.tile([128, 4, nblk], bf16, tag="fr")
            ring = pp.tile([128, 4, 133], bf16, tag="ring")
            ratio = pp.tile([128, 4, t_steps], bf16, tag="ratio")
            hcbuf = pp.tile([128, 4, t_steps], bf16, tag="hcbuf")
            edrev = pp.tile([128, 4, DMAX], bf16, tag="edrev")
            wexp = pp.tile([128, 2, 256], bf16, tag="wexp")
            ones_k = pp.tile([128, 1], bf16, tag="ones_k")
            ones_m = pp.tile([1, 128], fp32, tag="ones_m")
            gbuf = pp.tile([128, 4, t_steps], fp32, tag="gbuf")
            sout = pp.tile([1, 2, nblk], fp32, tag="sout")
            inv_s = pp.tile([1, 2], fp32, tag="inv_s")

            nc.sync.dma_start(ec1[:], ec1_d[:])
            nc.sync.dma_start(ec1x[:], ec1x_d[:])
            nc.sync.dma_start(sc[:], sc_d[:])
            nc.sync.dma_start(fr[:], fr_d[:])
            nc.sync.dma_start(ring[:], ring0_d[:])
            nc.sync.dma_start(ratio[:], ratio_d[:])
            nc.sync.dma_start(edrev[:], edrev_d[:])
            nc.sync.dma_start(wexp[:], wexp_d[:])
            nc.sync.dma_start(ones_k[:], ones_k_d[:])
            nc.sync.dma_start(ones_m[:], ones_m_d[:])
            nc.vector.memset(sout[:], 0.0)
            nc.vector.tensor_copy(hcbuf[:, :, 0], ring[:, :, 129])

            for t in range(t_steps):
                n, k = divmod(t, BLK)
                hslot = 131 + (t % 2)
                hcslot = 129 + (t % 2)
                # far duration filter (d=2..64) on GpSimd; window slots k..k+63
                prod = wp.tile([128, 4, DMAX], bf16, tag="prod")
                nc.gpsimd.tensor_mul(prod[:], ring[:, :, k : k + DMAX], edrev[:])
                nc.vector.tensor_reduce(
                    gbuf[:, :, t],
                    prod[:],
                    axis=mybir.AxisListType.X,
                    op=mybir.AluOpType.add,
                )
                # hfar column
                nc.vector.tensor_mul(ring[:, :, hslot], gbuf[:, :, t], ec1[:, :, t])
                # transition matvec: f = expA^T @ (hfar + hc), 8 matmuls
                pf = pfp.tile([128, 4], fp32, tag="pf")
                for hj in range(2):
                    for hi in range(2):
                        w_ap = wexp[:, hi, hj * 128 : (hj + 1) * 128]
                        nc.tensor.matmul(
                            pf[:, 2 * hj : 2 * hj + 2],
                            w_ap,
                            ring[:, 2 * hi : 2 * hi + 2, hslot],
                            start=(hi == 0),
                            stop=False,
                        )
                        nc.tensor.matmul(
                            pf[:, 2 * hj : 2 * hj + 2],
                            w_ap,
                            ring[:, 2 * hi : 2 * hi + 2, hcslot],
                            start=False,
                            stop=(hi == 1),
                        )
                # opA: w~ -> ring slot 65+k, hc for t+1 -> slot 129+((t+1)%2)
                base = 65 + k
                second = 129 + ((t + 1) % 2)
                out_ap = ring[:, :, base : second + 1 : second - base]
                nc.vector.tensor_mul(
                    out_ap,
                    pf[:].unsqueeze(2).broadcast_to([128, 4, 2]),
                    sc[:, :, t, :],
                )

                if k == PROBE_K and n < nblk - 1:
                    # probe: S[b] ~= sum_j hfar (both halves accumulate in psum)
                    ps = psp.tile([1, 2], fp32, tag="ps")
                    nc.tensor.matmul(
                        ps[:], ones_k[:], ring[:, 0:2, hslot], start=True, stop=False
                    )
                    nc.tensor.matmul(
                        ps[:], ones_k[:], ring[:, 2:4, hslot], start=False, stop=True
                    )
                    nc.vector.tensor_copy(sout[:, :, n], ps[0:1, :])
                    nc.vector.reciprocal(inv_s[:], ps[0:1, :])

                if k == BLK - 1:
                    # batched hc persistence for the output: slots 65..128
                    # hold w~ for tau = 64n..64n+63 -> hc for t = 64n+1..64n+64
                    wdt = min(BLK, t_steps - 1 - 64 * n)
                    nc.gpsimd.tensor_mul(
                        hcbuf[:, :, 64 * n + 1 : 64 * n + 1 + wdt],
                        ring[:, :, 65 : 65 + wdt],
                        ratio[:, :, 64 * n : 64 * n + wdt],
                    )

                if k == BLK - 1 and n < nblk - 1:
                    # block boundary: rescale+repack ring slots 64..128 -> 0..64
                    pb = psp.tile([128, 4], fp32, tag="pb")
                    rhs_b = inv_s[0:1, :].unsqueeze(1).broadcast_to([1, 2, 2])
                    nc.tensor.matmul(pb[:], ones_m[:], rhs_b, start=True, stop=True)
                    frmu = wp.tile([128, 4], bf16, tag="frmu")
                    nc.vector.tensor_mul(frmu[:], fr[:, :, n], pb[:])
                    nc.vector.tensor_mul(
                        ring[:, :, 0:65],
                        ring[:, :, 64:129],
                        frmu[:].unsqueeze(2).broadcast_to([128, 4, 65]),
                    )
                    nc.gpsimd.memset(ring[:, :, 65:129], 0.0)
                    # pending hc (slot 129) and its persisted copy: mu rescale
                    nc.vector.tensor_mul(ring[:, :, 129], ring[:, :, 129], pb[:])
                    nc.vector.tensor_mul(
                        hcbuf[:, :, 64 * (n + 1)], hcbuf[:, :, 64 * (n + 1)], pb[:]
                    )

            # alpha-centered output: G_far*EC1*2^-40 + hc*2^-40, then log
            nc.vector.tensor_mul(gbuf[:], gbuf[:], ec1x[:])
            nc.vector.scalar_tensor_tensor(
                gbuf[:],
                hcbuf[:],
                2.0**-40,
                gbuf[:],
                op0=mybir.AluOpType.mult,
                op1=mybir.AluOpType.add,
            )
            nc.scalar.activation(
                gbuf[:], gbuf[:], mybir.ActivationFunctionType.Ln
            )
            nc.sync.dma_start(araw_d[:], gbuf[:])
            nc.sync.dma_start(sprb_d[:], sout[:])

    nc.compile()
    return nc


# ---------------------------------------------------------------- golden numpy sim
def _golden(core_in, t_steps=T):
    """Replicates device ops (incl. bf16 rounding) for one core in numpy."""
    nblk = t_steps // BLK
    f32 = np.float32

    def bf(x):
        return x.astype(BF).astype(f32)

    ring = core_in["ring0"].astype(f32)  # [128, 4, 133]
    ec1x = core_in["ec1x"].astype(f32)
    edrev = core_in["edrev"].astype(f32)
    wexp = core_in["wexp"].astype(f32)
    ec1 = core_in["ec1"].astype(f32)
    sc = core_in["sc"].astype(f32)
    fr = core_in["fr"].astype(f32)
    ratio = core_in["ratio"].astype(f32)
    gbuf = np.zeros((128, 4, t_steps), f32)
    hcbuf = np.zeros((128, 4, t_steps), f32)
    hcbuf[:, :, 0] = ring[:, :, 129]
    sout = np.zeros((1, 2, nblk), f32)

    for t in range(t_steps):
        n, k = divmod(t, BLK)
        hslot = 131 + (t % 2)
        hcslot = 129 + (t % 2)
        prod = bf(ring[:, :, k : k + DMAX] * edrev)
        g = prod.sum(axis=2, dtype=f32)
        gbuf[:, :, t] = g
        hfar = bf(g * ec1[:, :, t])
        ring[:, :, hslot] = hfar
        hc = ring[:, :, hcslot]
        pf = np.zeros((128, 4), f32)
        for hj in range(2):
            for b in range(2):
                acc = np.zeros(128, f32)
                for hi in range(2):
                    w = wexp[:, hi, hj * 128 : (hj + 1) * 128]
                    acc += w.T @ hfar[:, 2 * hi + b] + w.T @ hc[:, 2 * hi + b]
                pf[:, 2 * hj + b] = acc
        ring[:, :, 65 + k] = bf(pf * sc[:, :, t, 0])
        ring[:, :, 129 + ((t + 1) % 2)] = bf(pf * sc[:, :, t, 1])
        if k == PROBE_K and n < nblk - 1:
            s4 = hfar.sum(axis=0, dtype=f32)
            sout[0, :, n] = s4[0:2] + s4[2:4]
        if k == BLK - 1:
            wdt = min(BLK, t_steps - 1 - 64 * n)
            hcbuf[:, :, 64 * n + 1 : 64 * n + 1 + wdt] = bf(
                ring[:, :, 65 : 65 + wdt] * ratio[:, :, 64 * n : 64 * n + wdt]
            )
        if k == BLK - 1 and n < nblk - 1:
            inv = (1.0 / sout[0, :, n]).astype(f32)
            frmu = bf(fr[:, :, n] * inv[None, [0, 1, 0, 1]])
            newhist = bf(ring[:, :, 64:129] * frmu[:, :, None])
            ring[:, :, 0:65] = newhist
            ring[:, :, 65:129] = 0.0
            ring[:, :, 129] = bf(ring[:, :, 129] * inv[None, [0, 1, 0, 1]])
            hcbuf[:, :, 64 * (n + 1)] = bf(
                hcbuf[:, :, 64 * (n + 1)] * inv[None, [0, 1, 0, 1]]
            )
    gtot = gbuf * ec1x + hcbuf * np.float32(2.0**-40)
    return np.log(gtot), sout


# ---------------------------------------------------------------- host postprocess
def _postprocess(raw_list, sp_list, t_steps=T):
    """raw: [128, 4, t] log(G*EC1*2^-40) per core; sp: [1, 2, nblk] probe sums."""
    nblk = t_steps // BLK
    shift = np.float32(40.0 * np.log(2.0))
    alphas = np.empty((B, t_steps, K), dtype=np.float32)
    for c in range(NCORES):
        raw = np.asarray(raw_list[c])  # [128, 4, t]
        sp = np.asarray(sp_list[c])  # [1, 2, nblk]
        logs = np.zeros((2, nblk), dtype=np.float64)
        logs[:, 1:] = np.log(np.maximum(sp[0, :, : nblk - 1], 1e-300))
        mu = (np.cumsum(logs, axis=1) + 40.0 * np.log(2.0)).astype(np.float32)
        nidx = np.arange(t_steps) // BLK
        for b in range(2):
            gb = SPC * c + b
            # [p, h, t] -> [t, h*128+p]
            lg = raw[:, [2 * 0 + b, 2 * 1 + b], :].transpose(2, 1, 0).reshape(t_steps, K)
            alphas[gb] = lg + mu[b, nidx][:, None]
    last = alphas[:, -1, :].astype(np.float64)
    m = last.max(axis=1)
    loglik = (np.log(np.exp(last - m[:, None]).sum(axis=1)) + m).astype(np.float32)
    return loglik, alphas


# ---------------------------------------------------------------- entry point
_CACHE = {}


def _run(log_B, pi_logits, A_logits, D_logits, trace=False, trace_kwargs=None):
    from concourse.bass_utils import run_bass_kernel_spmd

    t_steps = T
    per_core, host = _precompute(log_B, pi_logits, A_logits, D_logits, t_steps)
    if "nc" not in _CACHE:
        _CACHE["nc"] = _build(t_steps, debug=False)
    nc = _CACHE["nc"]
    in_maps = [per_core[c] for c in range(NCORES)]
    out = run_bass_kernel_spmd(
        nc, in_maps, list(range(NCORES)), trace=trace, **(trace_kwargs or {})
    )
    res = out.results
    raw_list = [res[c]["alpha_raw"] for c in range(NCORES)]
    sp_list = [res[c]["sprobe"] for c in range(NCORES)]
    loglik, alphas = _postprocess(raw_list, sp_list, t_steps)
    return loglik, alphas, out


def kernel(log_B, pi_logits, A_logits, D_logits):
    loglik, alphas, _ = _run(log_B, pi_logits, A_logits, D_logits)
    return loglik, alphas


def _ensure_ntff_hook():
    """Register the axon NTFF profile hook if the image lacks antenv.axon_hooks."""
    import types

    try:
        from antenv.axon_hooks import get_axon_ntff_profile_hook  # noqa: F401

        return
    except ImportError:
        pass
    if "/root/.axon_site" not in sys.path:
        sys.path.insert(0, "/root/.axon_site")
    from trn_agent_boot.trn_boot import _ntff_profile_via_ctypes

    import antenv

    hook = _ntff_profile_via_ctypes("/opt/axon/libaxon_pjrt.so")
    mod = types.ModuleType("antenv.axon_hooks")
    holder = {"h": hook}
    mod.get_axon_ntff_profile_hook = lambda: holder["h"]
    mod.set_axon_ntff_profile_hook = lambda h: holder.__setitem__("h", h)
    sys.modules["antenv.axon_hooks"] = mod
    antenv.axon_hooks = mod


def profile_exec_ns(log_B, pi_logits, A_logits, D_logits, tmpdir=None):
    """Run with NTFF tracing; returns HW exec time in ns (or None)."""
    _ensure_ntff_hook()
    from concourse import bass_utils as _bu

    if not getattr(_bu.upload_artifacts, "_patched", False):
        def _no_upload(tmpdir_):
            return "local://" + str(tmpdir_)

        _no_upload._patched = True
        _bu.upload_artifacts = _no_upload
    kw = {"tmpdir": tmpdir} if tmpdir else {}
    _, _, out = _run(log_B, pi_logits, A_logits, D_logits, trace=True, trace_kwargs=kw)
    return out.exec_time_ns


# revision 15
# speedup vs baseline: 1.7899x; 1.4346x over previous
"""HSMM forward-pass kernel for Trainium2 (8 NeuronCores, data-parallel over batch).

Algorithm: the explicit-duration HSMM forward recurrence is rewritten in
exp-space, where it becomes a *linear* recurrence with time-varying diagonal
scalings:

    w_t = EC2_t * ( expA^T @ (EC1_t * sum_{d=1..64} expD_d * w_{t-d}) )

with all EC* factors precomputable on the host from cumsum(log_B).  Numerical
range is controlled by per-j anchors Phi_n(j) = cumb at 64-step block
boundaries (host precomputed) plus one runtime scalar per block per sequence
(mu_n), estimated on-device with a partition-sum probe and applied as a ring
rescale at each block boundary.  alpha_t = log(hfar_t + hc1_t + hc2_t) + mu_n
is reconstructed at the end (log on device, + mu on host).

Each core handles 2 sequences; K=256 states are split into 2 partition-halves
(g = 2*h + b indexes the 4 (half, seq) groups laid out along the free dim).
The duration filter splits into a far part (d>=3, GpSimd products + Vector
reduce, 3-step pipelined lookahead) and near taps d=1,2 which ride as extra
PSUM-accumulated matmul columns using host-precomputed scales, so the serial
dependency cycle spans 3 steps.  The transition matvec runs on the Tensor
engine in bf16.

Ring layout (g-stride 265): slots 0..128 = w~ history (slot u <-> tau =
64n-65+u, new w~ at 65+k), 133+k = hc1 written at step t (consumed at t+1),
201+k = hc2 (consumed at t+2); opA writes all three with one stride-68 AP.
"""

import sys

import numpy as np

sys.path.insert(0, "/opt/trn_rl_repo")

import ml_dtypes

BF = ml_dtypes.bfloat16

B, T, K, DMAX = 16, 2048, 256, 64
NCORES = 8
SPC = B // NCORES  # sequences per core = 2
BLK = 64  # block length (= DMAX)
PROBE_K = 32  # probe offset within a block
RSLOTS = 265  # ring free size
LNSCALE = 2.0**-40  # scale inside the final Ln to stay in its input range


# ---------------------------------------------------------------- host precompute
def _precompute(log_B, pi_logits, A_logits, D_logits, t_steps=T):
    """Build per-core device input arrays."""
    nblk = t_steps // BLK
    logb = np.asarray(log_B, dtype=np.float64)[:, :t_steps, :]
    cumb = np.cumsum(logb, axis=1)  # [B, t, K] float64

    # anchors Phi_n = cumb at block starts (cumb_{64n-1}; 0 for n=0)
    phi = np.zeros((B, nblk, K), dtype=np.float64)
    for n in range(1, nblk):
        phi[:, n, :] = cumb[:, BLK * n - 1, :]

    nidx = np.arange(t_steps) // BLK
    phit = phi[:, nidx, :]  # [B, t, K]
    ec1 = np.exp(cumb - phit)  # [B, t, K]
    ec2 = np.exp(phit - cumb)
    fr = np.ones((B, nblk, K), dtype=np.float64)
    if nblk > 1:
        fr[:, : nblk - 1, :] = np.exp(phi[:, 1:, :] - phi[:, : nblk - 1, :])

    exppi = np.exp(np.asarray(pi_logits, dtype=np.float64))  # [K]
    expA = np.exp(np.asarray(A_logits, dtype=np.float64))  # [i, j]
    expD = np.exp(np.asarray(D_logits, dtype=np.float64))  # [j, d-1]
    expD1 = expD[:, 0]
    expD2 = expD[:, 1]

    # sc[b, t, j, :] = scales applied to f~_t: [w~_t, hc1_{t+1}, hc2_{t+2}]
    sc = np.zeros((B, t_steps, K, 3), dtype=np.float64)
    sc[..., 0] = ec2
    sc[:, : t_steps - 1, :, 1] = np.exp(logb[:, 1:, :]) * expD1[None, None, :]
    if t_steps >= 2:
        sc[:, : t_steps - 2, :, 2] = (
            np.exp(logb[:, 1:-1, :] + logb[:, 2:, :]) * expD2[None, None, :]
        )

    # ratioX[tau] recovers hcX_{tau+X} from stored w~_tau (batched per block)
    ratio1 = np.zeros((B, t_steps, K), dtype=np.float64)
    ratio1[:, : t_steps - 1, :] = (
        ec1[:, : t_steps - 1, :] * np.exp(logb[:, 1:, :]) * expD1[None, None, :]
    )
    ratio2 = np.zeros((B, t_steps, K), dtype=np.float64)
    if t_steps >= 3:
        ratio2[:, : t_steps - 2, :] = (
            ec1[:, : t_steps - 2, :]
            * np.exp(logb[:, 1:-1, :] + logb[:, 2:, :])
            * expD2[None, None, :]
        )

    # far filter weights: window slot k+1+u <-> d = 64-u, u in 0..61 (d>=3)
    edrev = np.zeros((128, 4, 62), dtype=np.float64)
    for h in range(2):
        for bq in range(2):
            edrev[:, 2 * h + bq, :] = expD[h * 128 : (h + 1) * 128, ::-1][:, 0:62]

    wexp = np.empty((128, 2, 256), dtype=np.float64)
    for hi in range(2):
        wexp[:, hi, :] = expA[hi * 128 : (hi + 1) * 128, :]

    ring0 = np.zeros((NCORES, 128, 4, RSLOTS), dtype=np.float64)
    for c in range(NCORES):
        for h in range(2):
            for bq in range(2):
                gb = SPC * c + bq
                hs = slice(h * 128, (h + 1) * 128)
                g = 2 * h + bq
                ring0[c, :, g, BLK] = exppi[hs]  # pi at tau=-1
                # hc1 for t=0 (slot 133+63) and hc2 for t=1 (slot 201+63)
                ring0[c, :, g, 196] = exppi[hs] * expD1[hs] * np.exp(cumb[gb, 0, hs])
                if t_steps >= 2:
                    ring0[c, :, g, 264] = (
                        exppi[hs] * expD2[hs] * np.exp(cumb[gb, 1, hs])
                    )

    def to_dev(x_btj, dtype):
        # [b, t, K] -> [128p, g=(h,b), t]
        b_, t_, _ = x_btj.shape
        v = x_btj.reshape(b_, t_, 2, 128).transpose(3, 2, 0, 1)  # [p, h, b, t]
        return np.ascontiguousarray(v.reshape(128, 2 * b_, t_)).astype(dtype)

    per_core = []
    for c in range(NCORES):
        bs = slice(SPC * c, SPC * (c + 1))
        scc = sc[bs].reshape(SPC, t_steps, 2, 128, 3).transpose(3, 2, 0, 1, 4)
        scc = np.ascontiguousarray(scc.reshape(128, 2 * SPC, t_steps, 3))
        per_core.append(
            dict(
                ec1=to_dev(ec1[bs], BF),
                sc=scc.astype(BF),
                ratio1=to_dev(ratio1[bs], BF),
                ratio2=to_dev(ratio2[bs], BF),
                fr=to_dev(fr[bs], BF),
                ring0=ring0[c].astype(BF),
                edrev=edrev.astype(BF),
                wexp=wexp.astype(BF),
                ones_k=np.ones((128, 1), dtype=BF),
                ones_m=np.ones((1, 128), dtype=np.float32),
            )
        )
    host = dict(nblk=nblk)
    return per_core, host


# ---------------------------------------------------------------- device kernel
def _build(t_steps=T, debug=False):
    import concourse.mybir as mybir
    from concourse import bacc, tile

    fp32 = mybir.dt.float32
    bf16 = mybir.dt.bfloat16
    nblk = t_steps // BLK
    ochunk = min(512, t_steps)

    nc = bacc.Bacc("TRN2", target_bir_lowering=False, debug=debug)

    ec1_d = nc.dram_tensor("ec1", [128, 4, t_steps], bf16, kind="ExternalInput")
    sc_d = nc.dram_tensor("sc", [128, 4, t_steps, 3], bf16, kind="ExternalInput")
    ratio1_d = nc.dram_tensor("ratio1", [128, 4, t_steps], bf16, kind="ExternalInput")
    ratio2_d = nc.dram_tensor("ratio2", [128, 4, t_steps], bf16, kind="ExternalInput")
    fr_d = nc.dram_tensor("fr", [128, 4, nblk], bf16, kind="ExternalInput")
    ring0_d = nc.dram_tensor("ring0", [128, 4, RSLOTS], bf16, kind="ExternalInput")
    edrev_d = nc.dram_tensor("edrev", [128, 4, 62], bf16, kind="ExternalInput")
    wexp_d = nc.dram_tensor("wexp", [128, 2, 256], bf16, kind="ExternalInput")
    ones_k_d = nc.dram_tensor("ones_k", [128, 1], bf16, kind="ExternalInput")
    ones_m_d = nc.dram_tensor("ones_m", [1, 128], fp32, kind="ExternalInput")

    araw_d = nc.dram_tensor("alpha_raw", [128, 4, t_steps], fp32, kind="ExternalOutput")
    sprb_d = nc.dram_tensor("sprobe", [1, 2, nblk], fp32, kind="ExternalOutput")

    with tile.TileContext(nc) as tc:
        with (
            tc.tile_pool(name="persist", bufs=1) as pp,
            tc.tile_pool(name="work", bufs=4) as wp,
            tc.tile_pool(name="outp", bufs=2) as op_,
            tc.tile_pool(name="psum", bufs=2, space="PSUM") as pfp,
            tc.tile_pool(name="psum_s", bufs=2, space="PSUM") as psp,
        ):
            ec1 = pp.tile([128, 4, t_steps], bf16, tag="ec1")
            sc = pp.tile([128, 4, t_steps, 3], bf16, tag="sc")
            ratio1 = pp.tile([128, 4, t_steps], bf16, tag="ratio1")
            ratio2 = pp.tile([128, 4, t_steps], bf16, tag="ratio2")
            fr = pp.tile([128, 4, nblk], bf16, tag="fr")
            ring = pp.tile([128, 4, RSLOTS], bf16, tag="ring")
            edrev = pp.tile([128, 4, 62], bf16, tag="edrev")
            wexp = pp.tile([128, 2, 256], bf16, tag="wexp")
            ones_k = pp.tile([128, 1], bf16, tag="ones_k")
            ones_m = pp.tile([1, 128], fp32, tag="ones_m")
            hfarb = pp.tile([128, 4, t_steps], bf16, tag="hfarb")
            hc1b = pp.tile([128, 4, t_steps], bf16, tag="hc1b")
            hc2b = pp.tile([128, 4, t_steps], bf16, tag="hc2b")
            sout = pp.tile([1, 2, nblk], fp32, tag="sout")
            inv_s = pp.tile([1, 2], fp32, tag="inv_s")

            nc.sync.dma_start(ec1[:], ec1_d[:])
            nc.sync.dma_start(sc[:], sc_d[:])
            nc.sync.dma_start(ratio1[:], ratio1_d[:])
            nc.sync.dma_start(ratio2[:], ratio2_d[:])
            nc.sync.dma_start(fr[:], fr_d[:])
            nc.sync.dma_start(ring[:], ring0_d[:])
            nc.sync.dma_start(edrev[:], edrev_d[:])
            nc.sync.dma_start(wexp[:], wexp_d[:])
            nc.sync.dma_start(ones_k[:], ones_k_d[:])
            nc.sync.dma_start(ones_m[:], ones_m_d[:])
            nc.vector.memset(sout[:], 0.0)
            # init: hc1_0, hc2_0 (=0), hc2_1 for the output buffers
            nc.vector.tensor_copy(hc1b[:, :, 0], ring[:, :, 196])
            nc.vector.memset(hc2b[:, :, 0:2], 0.0)
            if t_steps >= 2:
                nc.vector.tensor_copy(hc2b[:, :, 1], ring[:, :, 264])

            for t in range(t_steps):
                n, k = divmod(t, BLK)
                # far duration filter (d=3..64): window slots k+1..k+62
                prod = wp.tile([128, 4, 62], bf16, tag="prod")
                nc.gpsimd.tensor_mul(prod[:], ring[:, :, k + 1 : k + 63], edrev[:])
                gtmp = wp.tile([128, 4], fp32, tag="gtmp")
                nc.vector.tensor_reduce(
                    gtmp[:],
                    prod[:],
                    axis=mybir.AxisListType.X,
                    op=mybir.AluOpType.add,
                )
                # hfar column (persisted for the output)
                nc.vector.tensor_mul(hfarb[:, :, t], gtmp[:], ec1[:, :, t])
                # transition matvec on (hfar + hc1 + hc2): 12 matmuls
                k1 = (t - 1) % BLK
                k2 = (t - 2) % BLK
                pf = pfp.tile([128, 4], fp32, tag="pf")
                for hj in range(2):
                    for hi in range(2):
                        w_ap = wexp[:, hi, hj * 128 : (hj + 1) * 128]
                        gsl = slice(2 * hi, 2 * hi + 2)
                        nc.tensor.matmul(
                            pf[:, 2 * hj : 2 * hj + 2],
                            w_ap,
                            hfarb[:, gsl, t],
                            start=(hi == 0),
                            stop=False,
                        )
                        nc.tensor.matmul(
                            pf[:, 2 * hj : 2 * hj + 2],
                            w_ap,
                            ring[:, gsl, 201 + k2],
                            start=False,
                            stop=False,
                        )
                        nc.tensor.matmul(
                            pf[:, 2 * hj : 2 * hj + 2],
                            w_ap,
                            ring[:, gsl, 133 + k1],
                            start=False,
                            stop=(hi == 1),
                        )
                # opA: [w~_t, hc1_{t+1}, hc2_{t+2}] at slots 65+k, 133+k, 201+k
                nc.vector.tensor_mul(
                    ring[:, :, 65 + k : 202 + k : 68],
                    pf[:].unsqueeze(2).broadcast_to([128, 4, 3]),
                    sc[:, :, t, :],
                )

                if k == PROBE_K and n < nblk - 1:
                    # probe: S[b] ~= sum_j hfar (halves accumulate in psum)
                    ps = psp.tile([1, 2], fp32, tag="ps")
                    nc.tensor.matmul(
                        ps[:], ones_k[:], hfarb[:, 0:2, t], start=True, stop=False
                    )
                    nc.tensor.matmul(
                        ps[:], ones_k[:], hfarb[:, 2:4, t], start=False, stop=True
                    )
                    nc.vector.tensor_copy(sout[:, :, n], ps[0:1, :])
                    nc.vector.reciprocal(inv_s[:], ps[0:1, :])

                if k == BLK - 1:
                    # batched hc persistence from this block's stored w~'s
                    w1 = min(BLK, t_steps - 1 - BLK * n)
                    if w1 > 0:
                        nc.gpsimd.tensor_mul(
                            hc1b[:, :, BLK * n + 1 : BLK * n + 1 + w1],
                            ring[:, :, 65 : 65 + w1],
                            ratio1[:, :, BLK * n : BLK * n + w1],
                        )
                    w2 = min(BLK, t_steps - 2 - BLK * n)
                    if w2 > 0:
                        nc.gpsimd.tensor_mul(
                            hc2b[:, :, BLK * n + 2 : BLK * n + 2 + w2],
                            ring[:, :, 65 : 65 + w2],
                            ratio2[:, :, BLK * n : BLK * n + w2],
                        )

                if k == BLK - 1 and n < nblk - 1:
                    # block boundary: mu rescale + repack slots 64..128 -> 0..64
                    pb = psp.tile([128, 4], fp32, tag="pb")
                    rhs_b = inv_s[0:1, :].unsqueeze(1).broadcast_to([1, 2, 2])
                    nc.tensor.matmul(pb[:], ones_m[:], rhs_b, start=True, stop=True)
                    frmu = wp.tile([128, 4], bf16, tag="frmu")
                    nc.vector.tensor_mul(frmu[:], fr[:, :, n], pb[:])
                    nc.vector.tensor_mul(
                        ring[:, :, 0:65],
                        ring[:, :, 64:129],
                        frmu[:].unsqueeze(2).broadcast_to([128, 4, 65]),
                    )
                    nc.gpsimd.memset(ring[:, :, 65:129], 0.0)
                    # pending hc's at old mu: ring slots 196 and 263..264,
                    # plus their persisted copies
                    nc.vector.tensor_mul(ring[:, :, 196], ring[:, :, 196], pb[:])
                    nc.vector.tensor_mul(
                        ring[:, :, 263:265],
                        ring[:, :, 263:265],
                        pb[:].unsqueeze(2).broadcast_to([128, 4, 2]),
                    )
                    nc.vector.tensor_mul(
                        hc1b[:, :, BLK * (n + 1)], hc1b[:, :, BLK * (n + 1)], pb[:]
                    )
                    nc.vector.tensor_mul(
                        hc2b[:, :, BLK * (n + 1) : BLK * (n + 1) + 2],
                        hc2b[:, :, BLK * (n + 1) : BLK * (n + 1) + 2],
                        pb[:].unsqueeze(2).broadcast_to([128, 4, 2]),
                    )

            # output: alpha_raw = log((hfar + hc1 + hc2) * 2^-40), chunked
            for c0 in range(0, t_steps, ochunk):
                csl = slice(c0, c0 + ochunk)
                s1 = op_.tile([128, 4, ochunk], fp32, tag="s1")
                nc.vector.tensor_add(s1[:], hc1b[:, :, csl], hc2b[:, :, csl])
                nc.vector.tensor_add(s1[:], s1[:], hfarb[:, :, csl])
                nc.scalar.activation(
                    s1[:], s1[:], mybir.ActivationFunctionType.Ln, scale=LNSCALE
                )
                nc.sync.dma_start(araw_d[:, :, csl], s1[:])
            nc.sync.dma_start(sprb_d[:], sout[:])

    nc.compile()
    return nc


# ---------------------------------------------------------------- golden numpy sim
def _golden(core_in, t_steps=T):
    """Replicates device ops (incl. bf16 rounding) for one core in numpy."""
    nblk = t_steps // BLK
    f32 = np.float32

    def bf(x):
        return x.astype(BF).astype(f32)

    ring = core_in["ring0"].astype(f32)  # [128, 4, RSLOTS]
    edrev = core_in["edrev"].astype(f32)
    wexp = core_in["wexp"].astype(f32)
    ec1 = core_in["ec1"].astype(f32)
    sc = core_in["sc"].astype(f32)
    fr = core_in["fr"].astype(f32)
    ratio1 = core_in["ratio1"].astype(f32)
    ratio2 = core_in["ratio2"].astype(f32)
    hfarb = np.zeros((128, 4, t_steps), f32)
    hc1b = np.zeros((128, 4, t_steps), f32)
    hc2b = np.zeros((128, 4, t_steps), f32)
    hc1b[:, :, 0] = ring[:, :, 196]
    if t_steps >= 2:
        hc2b[:, :, 1] = ring[:, :, 264]
    sout = np.zeros((1, 2, nblk), f32)

    for t in range(t_steps):
        n, k = divmod(t, BLK)
        prod = bf(ring[:, :, k + 1 : k + 63] * edrev)
        g = prod.sum(axis=2, dtype=f32)
        hfar = bf(g * ec1[:, :, t])
        hfarb[:, :, t] = hfar
        k1 = (t - 1) % BLK
        k2 = (t - 2) % BLK
        hc1 = ring[:, :, 133 + k1]
        hc2 = ring[:, :, 201 + k2]
        pf = np.zeros((128, 4), f32)
        for hj in range(2):
            for bq in range(2):
                acc = np.zeros(128, f32)
                for hi in range(2):
                    w = wexp[:, hi, hj * 128 : (hj + 1) * 128]
                    gi = 2 * hi + bq
                    acc += w.T @ (hfar[:, gi] + hc1[:, gi] + hc2[:, gi])
                pf[:, 2 * hj + bq] = acc
        ring[:, :, 65 + k] = bf(pf * sc[:, :, t, 0])
        ring[:, :, 133 + k] = bf(pf * sc[:, :, t, 1])
        ring[:, :, 201 + k] = bf(pf * sc[:, :, t, 2])
        if k == PROBE_K and n < nblk - 1:
            s4 = hfar.sum(axis=0, dtype=f32)
            sout[0, :, n] = s4[0:2] + s4[2:4]
        if k == BLK - 1:
            w1 = min(BLK, t_steps - 1 - BLK * n)
            if w1 > 0:
                hc1b[:, :, BLK * n + 1 : BLK * n + 1 + w1] = bf(
                    ring[:, :, 65 : 65 + w1] * ratio1[:, :, BLK * n : BLK * n + w1]
                )
            w2 = min(BLK, t_steps - 2 - BLK * n)
            if w2 > 0:
                hc2b[:, :, BLK * n + 2 : BLK * n + 2 + w2] = bf(
                    ring[:, :, 65 : 65 + w2] * ratio2[:, :, BLK * n : BLK * n + w2]
                )
        if k == BLK - 1 and n < nblk - 1:
            inv = (1.0 / sout[0, :, n]).astype(f32)[[0, 1, 0, 1]]
            frmu = bf(fr[:, :, n] * inv[None, :])
            ring[:, :, 0:65] = bf(ring[:, :, 64:129] * frmu[:, :, None])
            ring[:, :, 65:129] = 0.0
            ring[:, :, 196] = bf(ring[:, :, 196] * inv[None, :])
            ring[:, :, 263:265] = bf(ring[:, :, 263:265] * inv[None, :, None])
            hc1b[:, :, BLK * (n + 1)] = bf(hc1b[:, :, BLK * (n + 1)] * inv[None, :])
            hc2b[:, :, BLK * (n + 1) : BLK * (n + 1) + 2] = bf(
                hc2b[:, :, BLK * (n + 1) : BLK * (n + 1) + 2] * inv[None, :, None]
            )
    tot = (hfarb + hc1b + hc2b).astype(f32)
    return np.log(tot * np.float32(LNSCALE)), sout


# ---------------------------------------------------------------- host postprocess
def _postprocess(raw_list, sp_list, t_steps=T):
    """raw: [128, 4, t] log((hfar+hc)*2^-40) per core; sp: probe sums."""
    nblk = t_steps // BLK
    alphas = np.empty((B, t_steps, K), dtype=np.float32)
    for c in range(NCORES):
        raw = np.asarray(raw_list[c])  # [128, 4, t]
        sp = np.asarray(sp_list[c])  # [1, 2, nblk]
        logs = np.zeros((2, nblk), dtype=np.float64)
        logs[:, 1:] = np.log(np.maximum(sp[0, :, : nblk - 1], 1e-300))
        mu = (np.cumsum(logs, axis=1) - np.log(LNSCALE)).astype(np.float32)
        nidx = np.arange(t_steps) // BLK
        for bq in range(2):
            gb = SPC * c + bq
            lg = raw[:, [bq, 2 + bq], :].transpose(2, 1, 0).reshape(t_steps, K)
            alphas[gb] = lg + mu[bq, nidx][:, None]
    last = alphas[:, -1, :].astype(np.float64)
    m = last.max(axis=1)
    loglik = (np.log(np.exp(last - m[:, None]).sum(axis=1)) + m).astype(np.float32)
    return loglik, alphas


# ---------------------------------------------------------------- entry point
_CACHE = {}


def _run(log_B, pi_logits, A_logits, D_logits, trace=False, trace_kwargs=None):
    from concourse.bass_utils import run_bass_kernel_spmd

    t_steps = T
    per_core, host = _precompute(log_B, pi_logits, A_logits, D_logits, t_steps)
    if "nc" not in _CACHE:
        _CACHE["nc"] = _build(t_steps, debug=False)
    nc = _CACHE["nc"]
    in_maps = [per_core[c] for c in range(NCORES)]
    out = run_bass_kernel_spmd(
        nc, in_maps, list(range(NCORES)), trace=trace, **(trace_kwargs or {})
    )
    res = out.results
    raw_list = [res[c]["alpha_raw"] for c in range(NCORES)]
    sp_list = [res[c]["sprobe"] for c in range(NCORES)]
    loglik, alphas = _postprocess(raw_list, sp_list, t_steps)
    return loglik, alphas, out


def kernel(log_B, pi_logits, A_logits, D_logits):
    loglik, alphas, _ = _run(log_B, pi_logits, A_logits, D_logits)
    return loglik, alphas


def _ensure_ntff_hook():
    """Register the axon NTFF profile hook if the image lacks antenv.axon_hooks."""
    import types

    try:
        from antenv.axon_hooks import get_axon_ntff_profile_hook  # noqa: F401

        return
    except ImportError:
        pass
    if "/root/.axon_site" not in sys.path:
        sys.path.insert(0, "/root/.axon_site")
    from trn_agent_boot.trn_boot import _ntff_profile_via_ctypes

    import antenv

    hook = _ntff_profile_via_ctypes("/opt/axon/libaxon_pjrt.so")
    mod = types.ModuleType("antenv.axon_hooks")
    holder = {"h": hook}
    mod.get_axon_ntff_profile_hook = lambda: holder["h"]
    mod.set_axon_ntff_profile_hook = lambda h: holder.__setitem__("h", h)
    sys.modules["antenv.axon_hooks"] = mod
    antenv.axon_hooks = mod


def profile_exec_ns(log_B, pi_logits, A_logits, D_logits, tmpdir=None):
    """Run with NTFF tracing; returns HW exec time in ns (or None)."""
    _ensure_ntff_hook()
    from concourse import bass_utils as _bu

    if not getattr(_bu.upload_artifacts, "_patched", False):

        def _no_upload(tmpdir_):
            return "local://" + str(tmpdir_)

        _no_upload._patched = True
        _bu.upload_artifacts = _no_upload
    kw = {"tmpdir": tmpdir} if tmpdir else {}
    _, _, out = _run(log_B, pi_logits, A_logits, D_logits, trace=True, trace_kwargs=kw)
    return out.exec_time_ns
